# revision 13
# baseline (speedup 1.0000x reference)
"""Trainium2 Bass kernel for nn_KMLoss (segment_reduce proto-network loss).

Math (exact decomposition of the reference):
  L[q,s] = 0.5*|xq_q - xs_s|^2, logit = -L.  All device values drop the
  per-query term 0.5|xq|^2 (it cancels in neg-pos) and are shifted by global
  constants so fp32 exp never over/underflows:
    pos path:  P[q,j] = (0.5|xs_j|^2 - SHP) - xq.xs_j      (same-class j)
    neg path:  A[q,c] = (0.5*S2_c/cnt_c - SA) - xq.(T_c/cnt_c)
  The own-class column of A gets a per-query correction `fix` (removes the
  self entry and applies the -INF mask + cnt-1 denominator), folded into the
  matmul as an extra contraction row times an indicator column.  The device
  returns per-row exp-sums only; host takes logs, subtracts the (fp8-identical
  host-simulated) self column from the pos sum, and averages.

Per core: 8 class-blocks (core k owns classes ranked {b*8+k} by size).  Block
b owns PSUM bank b: a K=4 bf16 aug matmul (column constants + fix row; 4-way
row-tiled, start=True, runs while inputs stream) + two K=128 fp8 dot matmuls.
3 input DMAs (aug + two merged pair loads, one issued from the ACT ring to
parallelize HWDGE descriptor generation), dummy matmuls to lift the PE HAM
throttle during the load, chunked exp (ACT) + sum-reduce (DVE) sized to keep
the tail short (last block's sums via the ACT accumulator), 2 output DMAs.
"""

import sys

import numpy as np

sys.path.insert(0, "/opt/trn_rl_repo")

NCORES = 8
C = 64
CPB = C // NCORES
D = 256
INF = 1000.0

# exp/reduce chunking of the 8 blocks; last chunk's sums use ACT accum
CHUNKS = [(0, 2), (2, 4), (4, 6), (6, 7), (7, 8)]
PS_BANKS = [2, 2, 2, 1, 1]  # PSUM banks per chunk (sum = 8)
N_WARM_MM = 8

_PROGRAM_CACHE = {}


def _round_up(x, m):
    return -(-x // m) * m


def _build_program(QC, WS):
    import concourse.bacc as bacc
    import concourse.tile as tile
    from concourse import mybir

    dt = mybir.dt
    Alu = mybir.AluOpType
    Act = mybir.ActivationFunctionType
    f8 = dt.float8e4
    NC = [w + C for w in WS]

    nc = bacc.Bacc(
        "TRN2",
        target_bir_lowering=False,
        debug=False,
        enable_asserts=False,
        num_devices=NCORES,
    )

    # aug: row-group g in {0,1} (partitions 32g..32g+3) holds blocks with
    # b%2==g: [lhs x4 | rhs x4], zero-padded to uniform AUGW cols
    grp = [[b for b in range(8) if b % 2 == g] for g in range(2)]
    aug_cols = [4 * QC + sum(NC[b] for b in g) for g in grp]
    AUGW = max(aug_cols)
    aug_d = nc.dram_tensor("aug", [36, AUGW], dt.bfloat16, kind="ExternalInput").ap()
    pair_cols = [4 * QC + 2 * NC[2 * p] + 2 * NC[2 * p + 1] for p in range(4)]
    pair_d = [nc.dram_tensor(f"pair{p}", [128, pair_cols[p]], f8,
                             kind="ExternalInput").ap() for p in range(4)]
    out_d = nc.dram_tensor("out", [QC, 16], dt.float32, kind="ExternalOutput").ap()

    with tile.TileContext(nc) as tc:
        with (
            tc.tile_pool(name="io", bufs=1) as io,
            tc.tile_pool(name="pp", bufs=1, space="PSUM") as pp,
        ):
            warm_i = io.tile([128, 16], dt.float32)
            warm_o = io.tile([128, 16], dt.float32)
            w_s = io.tile([128, 128], dt.bfloat16)
            t_aug = io.tile([128, AUGW], dt.bfloat16)
            t_pair = [io.tile([128, pair_cols[p]], f8, name=f"t_pair{p}")
                      for p in range(4)]
            e_t = [io.tile([128, (hi - lo) * NC[lo]], dt.float32, name=f"e_t{lo}")
                   for lo, hi in CHUNKS]
            out0 = io.tile([128, 8], dt.float32)
            out1 = io.tile([128, 8], dt.float32)
            ps_t = [pp.tile([128, 512 * nb], dt.float32, name=f"ps_c{i}")
                    for i, nb in enumerate(PS_BANKS)]

            # scratch init on gpsimd (keeps DVE free)
            nc.gpsimd.memset(warm_i[:, :], 0.0)
            nc.gpsimd.memset(w_s[:, :], 0.0)

            # input DMAs split across the two HWDGE rings.  The ACT exp-table
            # load + warm-up activation sit between the aug and pair2/3 DMA
            # issues on the scalar ring: the ~1.3us table load delays the late
            # pairs' packets so aug/pair0 completion isn't starved of SDMA
            # engine time by data that is only needed later.
            nc.scalar.dma_start(out=t_aug[0:36, :], in_=aug_d)
            nc.sync.dma_start(out=t_pair[0], in_=pair_d[0])
            nc.sync.dma_start(out=t_pair[1], in_=pair_d[1])
            nc.scalar.activation(warm_o[:, :], warm_i[:, :], Act.Exp)
            nc.scalar.dma_start(out=t_pair[2], in_=pair_d[2])
            nc.scalar.dma_start(out=t_pair[3], in_=pair_d[3])

            # dummy matmuls: lift the PE HAM clock gate while inputs stream
            for _ in range(N_WARM_MM):
                nc.tensor.matmul(
                    ps_t[0][0:128, 0:128], w_s[:, 0:128], w_s[:, 0:128],
                    start=True, stop=True,
                )

            def chunk_of(b):
                for ci, (lo, hi) in enumerate(CHUNKS):
                    if lo <= b < hi:
                        return ci, b - lo
                raise AssertionError

            def bank(b):
                ci, off = chunk_of(b)
                return ps_t[ci][0:QC, off * 512:off * 512 + NC[b]]

            # aug matmuls open each bank's accumulation group (2-way row tiled)
            for b in range(8):
                g = b % 2
                gi = grp[g].index(b)
                la = gi * QC
                ra = 4 * QC + sum(NC[x] for x in grp[g][:gi])
                nc.tensor.matmul(
                    bank(b),
                    t_aug[32 * g:32 * g + 4, la:la + QC],
                    t_aug[32 * g:32 * g + 4, ra:ra + NC[b]],
                    start=True,
                    stop=False,
                    tile_position=(32 * g, 0),
                )

            def emit_dots(b):
                p, s = b // 2, b % 2
                t_in = t_pair[p]
                rbase = 4 * QC + s * 2 * NC[b]
                lhsT = t_in[:, s * 2 * QC:(s + 1) * 2 * QC].rearrange(
                    "p (two f) -> p two f", two=2)
                rhs = t_in[:, rbase:rbase + 2 * NC[b]].rearrange(
                    "p (two f) -> p two f", two=2)
                nc.tensor.matmul(
                    bank(b), lhsT, rhs, start=False, stop=True,
                    perf_mode=mybir.MatmulPerfMode.DoubleRow,
                )

            def emit_chunk(ci):
                lo, hi = CHUNKS[ci]
                n = hi - lo
                W, NCb = WS[lo], NC[lo]
                ps = ps_t[ci]
                if n > 1:
                    src = ps[0:QC, 0:n * 512].rearrange(
                        "p (g c) -> p g c", g=n)[:, :, 0:NCb]
                    dst = e_t[ci][0:QC, :].rearrange("p (g c) -> p g c", c=NCb)
                else:
                    src = ps[0:QC, 0:NCb]
                    dst = e_t[ci][0:QC, :]
                ot = out0 if lo < 4 else out1
                oc = lo % 4
                if ci == len(CHUNKS) - 1:
                    # single block: exp pos / agg separately, sums via ACT accum
                    nc.scalar.activation(
                        e_t[ci][0:QC, 0:W], ps[0:QC, 0:W], Act.Exp, scale=-1.0,
                        accum_out=ot[0:QC, oc:oc + 1],
                    )
                    nc.scalar.activation(
                        e_t[ci][0:QC, W:NCb], ps[0:QC, W:NCb], Act.Exp, scale=-1.0,
                        accum_out=ot[0:QC, 4 + oc:4 + oc + 1],
                    )
                    return
                nc.scalar.activation(dst, src, Act.Exp, scale=-1.0)
                ev = e_t[ci][0:QC, :].rearrange("p (g c) -> p g c", c=NCb)
                nc.vector.tensor_reduce(
                    out=ot[0:QC, oc:oc + n], in_=ev[:, :, 0:W],
                    axis=mybir.AxisListType.X, op=Alu.add,
                )
                nc.vector.tensor_reduce(
                    out=ot[0:QC, 4 + oc:4 + oc + n], in_=ev[:, :, W:NCb],
                    axis=mybir.AxisListType.X, op=Alu.add,
                )

            for b in range(2):
                emit_dots(b)
            emit_chunk(0)
            for b in range(2, 4):
                emit_dots(b)
            emit_chunk(1)
            nc.sync.dma_start(out=out_d[:, 0:8], in_=out0[0:QC, :])
            for b in range(4, 6):
                emit_dots(b)
            emit_chunk(2)
            emit_dots(6)
            emit_chunk(3)
            emit_dots(7)
            emit_chunk(4)
            nc.sync.dma_start(out=out_d[:, 8:16], in_=out1[0:QC, :])

    nc.compile()
    return nc


def _prepare(xq, yq, xs, ys, pos):
    """Host-side prep: class stats, shifts, per-core packed arrays."""
    import ml_dtypes

    bf16 = ml_dtypes.bfloat16
    f8 = ml_dtypes.float8_e4m3
    Nq = xq.shape[0]
    xq64 = xq.astype(np.float64)
    xs64 = xs.astype(np.float64)

    cnt = np.bincount(ys, minlength=C).astype(np.float64)
    assert (cnt > 1).all(), "singleton/empty classes unsupported by this kernel"
    T_c = np.zeros((C, D))
    np.add.at(T_c, ys, xs64)
    S2_c = np.zeros(C)
    np.add.at(S2_c, ys, (xs64 ** 2).sum(-1))
    xs2h = 0.5 * (xs64 ** 2).sum(-1)
    xq2h = 0.5 * (xq64 ** 2).sum(-1)
    agg_scaled = T_c / cnt[:, None]

    A_exact = 0.5 * S2_c[None, :] / cnt[None, :] - xq.astype(np.float32) @ \
        agg_scaled.T.astype(np.float32)
    SA = float(np.round(np.median(A_exact)))
    assert np.abs(A_exact - SA).max() < 70, "neg-path shift margin exceeded"

    sidx = [np.where(ys == c)[0] for c in range(C)]
    qidx = [np.where(yq == c)[0] for c in range(C)]
    xsf = xs.astype(np.float32)
    xqf = xq.astype(np.float32)
    pm_lo, pm_hi = 1e30, -1e30
    for c in range(C):
        if len(qidx[c]) == 0 or len(sidx[c]) == 0:
            continue
        P = xs2h[sidx[c]][None, :].astype(np.float32) - xqf[qidx[c]] @ xsf[sidx[c]].T
        m = P.min(axis=1)
        pm_lo = min(pm_lo, float(m.min()))
        pm_hi = max(pm_hi, float(m.max()))
    assert pm_hi - pm_lo < 150, "pos-path shift window too wide"
    SHP = float(np.round((pm_lo + pm_hi) / 2))

    H = 0.5 * S2_c[yq] - np.einsum('qd,qd->q', xq64, T_c[yq])
    Lself_t = xs2h[pos] - np.einsum('qd,qd->q', xq64, xs64[pos])
    adj = cnt[yq] - 1.0
    fix = xq2h / adj + H / (cnt[yq] * adj) + (INF - (Lself_t + xq2h)) / adj

    def hilo(v):
        hi = v.astype(bf16).astype(np.float64)
        lo = (v - hi).astype(bf16)
        return hi.astype(bf16), lo

    fix_hi, fix_lo = hilo(fix)
    cpos_hi, cpos_lo = hilo(xs2h - SHP)
    cagg_hi, cagg_lo = hilo(0.5 * S2_c / cnt - SA)
    cpos_dev = cpos_hi.astype(np.float64) + cpos_lo.astype(np.float64)

    order = np.argsort(-cnt, kind='stable')
    QC = _round_up(max(max(len(q) for q in qidx), 1), 16)
    WS = [0] * 8
    for b in range(8):
        WS[b] = _round_up(max(len(sidx[order[b * 8 + k]]) for k in range(NCORES)), 16)
    for lo, hi in CHUNKS:
        w = max(WS[lo:hi])
        for b in range(lo, hi):
            WS[b] = w
    NC = [w + C for w in WS]
    assert max(NC) <= 512 and QC <= 128

    xq_r8 = (-xqf).astype(f8)
    xs_r8 = xsf.astype(f8)
    agg_r8 = agg_scaled.astype(np.float32).astype(f8)

    grp = [[b for b in range(8) if b % 2 == g] for g in range(2)]
    aug_cols = [4 * QC + sum(NC[b] for b in g) for g in grp]
    AUGW = max(aug_cols)
    pair_cols = [4 * QC + 2 * NC[2 * p] + 2 * NC[2 * p + 1] for p in range(4)]

    in_maps = []
    meta = []
    for k in range(NCORES):
        aug = np.zeros((36, AUGW), np.float32)
        pairs = [np.zeros((128, pair_cols[p]), f8) for p in range(4)]
        core_meta = []
        for b in range(8):
            cls = int(order[b * 8 + k])
            qi = qidx[cls]
            si = sidx[cls]
            nq, ns = len(qi), len(si)
            W, NCb = WS[b], NC[b]
            g = b % 2
            gi = grp[g].index(b)
            la = gi * QC
            ra = 4 * QC + sum(NC[x] for x in grp[g][:gi])
            gp = 32 * g
            aug[gp + 0, la:la + nq] = 1.0
            aug[gp + 1, la:la + nq] = 1.0
            aug[gp + 2, la:la + nq] = fix_hi[qi].astype(np.float32)
            aug[gp + 3, la:la + nq] = fix_lo[qi].astype(np.float32)
            aug[gp + 0, ra:ra + ns] = cpos_hi[si].astype(np.float32)
            aug[gp + 1, ra:ra + ns] = cpos_lo[si].astype(np.float32)
            aug[gp + 0, ra + ns:ra + W] = 2000.0
            aug[gp + 0, ra + W:ra + NCb] = cagg_hi.astype(np.float32)
            aug[gp + 1, ra + W:ra + NCb] = cagg_lo.astype(np.float32)
            aug[gp + 2, ra + W + cls] = 1.0
            aug[gp + 3, ra + W + cls] = 1.0
            p, s = b // 2, b % 2
            P = pairs[p]
            for h in range(2):
                rows = slice(h * 128, (h + 1) * 128)
                P[:, s * 2 * QC + h * QC:s * 2 * QC + h * QC + nq] = xq_r8[qi].T[rows]
                rb = 4 * QC + s * 2 * NCb + h * NCb
                P[:, rb:rb + ns] = xs_r8[si].T[rows]
                P[:, rb + W:rb + NCb] = agg_r8.T[rows]
            core_meta.append((cls, qi))
        im = {"aug": aug.astype(bf16)}
        for p in range(4):
            im[f"pair{p}"] = pairs[p]
        in_maps.append(im)
        meta.append(core_meta)

    host = {
        "SHP": SHP, "SA": SA,
        "tdev": np.exp(-((np.einsum('qd,qd->q',
                                    xq_r8.astype(np.float32),
                                    xs_r8[pos].astype(np.float32))
                          ).astype(np.float32).astype(np.float64)
                         + cpos_dev[pos])),
    }
    return QC, WS, in_maps, meta, host, Nq


def _reduce_host(results, meta, host, Nq):
    SHP, SA = host["SHP"], host["SA"]
    total = 0.0
    for k in range(NCORES):
        o = np.asarray(results[k]["out"], np.float64)
        for b, (cls, qi) in enumerate(meta[k]):
            n = len(qi)
            if n == 0:
                continue
            scol = b if b < 4 else 4 + b
            S = o[0:n, scol]
            Sn = o[0:n, scol + 4]
            S_excl = np.maximum(S - host["tdev"][qi], S * 1e-7)
            pos_v = np.log(S_excl) - SHP
            neg_v = np.log(Sn) - SA
            total += (neg_v - pos_v).sum()
    return np.array(total / Nq, dtype=np.float32)


def _run(xq, yq, xs, ys, pos, trace=False, tmpdir=None):
    from concourse import bass_utils

    xq = np.ascontiguousarray(np.asarray(xq, np.float32))
    xs = np.ascontiguousarray(np.asarray(xs, np.float32))
    yq = np.asarray(yq).astype(np.int64)
    ys = np.asarray(ys).astype(np.int64)
    pos = np.asarray(pos).astype(np.int64)

    QC, WS, in_maps, meta, host, Nq = _prepare(xq, yq, xs, ys, pos)
    key = (QC, tuple(WS))
    if key not in _PROGRAM_CACHE:
        _PROGRAM_CACHE[key] = _build_program(QC, WS)
    nc = _PROGRAM_CACHE[key]

    kw = {}
    if trace:
        kw = dict(trace=True, tmpdir=tmpdir)
    res = bass_utils.run_bass_kernel_spmd(
        nc, in_maps, core_ids=list(range(NCORES)), **kw
    )
    return _reduce_host(res.results, meta, host, Nq), res


def kernel(xq, yq, xs, ys, pos):
    loss, _ = _run(xq, yq, xs, ys, pos, trace=False)
    return loss
